# revision 4
# baseline (speedup 1.0000x reference)
"""CN2LinkPredictor kernel for 8 Trainium2 NeuronCores.

Strategy:
  Phase A (device): A2 = (A_u @ adj) > 0 for the unique edge-endpoint rows
    A_u of adj, sharded across 8 cores, fp8 matmul (exact: 0/1 inputs with
    fp32 PSUM accumulation) with a >0.5 threshold on PSUM eviction.
  Host: gather/transpose per-edge mask rows (sharding/data-distribution only).
  Phase B (device): per-edge masked neighborhood sums (4 mask products on
    DVE, matmuls against x on PE) + all MLPs, edges sharded across 8 cores,
    computed in transposed layout [channels, edges].

All FLOPs (matmuls, mask intersections, MLPs, thresholds) run on device.
Host does sharding, gathers, transposes, dtype casts and weight scaling
(folding the cumprod(sigmoid(alpha)) / beta scalars into the last-layer
weights of each branch).
"""
import sys
import types

import numpy as np
import ml_dtypes

BF16 = ml_dtypes.bfloat16
FP8 = ml_dtypes.float8_e4m3
ONE_FP8 = np.uint8(0x38)  # bit pattern of 1.0 in float8_e4m3

N = 8192
IN_CH = 128
HID = 256
E = 4096
NCORES = 8
E_CORE = E // NCORES
P = 128
KO = N // P


def _install_hooks():
    """Make antenv.axon_hooks importable so traced runs work (optional)."""
    if "antenv.axon_hooks" in sys.modules:
        return
    try:
        import antenv
        mod = types.ModuleType("antenv.axon_hooks")
        mod._hook = None

        def set_axon_ntff_profile_hook(h):
            mod._hook = h

        def get_axon_ntff_profile_hook():
            return mod._hook

        mod.set_axon_ntff_profile_hook = set_axon_ntff_profile_hook
        mod.get_axon_ntff_profile_hook = get_axon_ntff_profile_hook
        sys.modules["antenv.axon_hooks"] = mod
        antenv.axon_hooks = mod
        try:
            from trn_agent_boot.trn_boot import _ntff_profile_via_ctypes
            hook = _ntff_profile_via_ctypes("/opt/axon/libaxon_pjrt.so")
            if hook is not None:
                set_axon_ntff_profile_hook(hook)
        except Exception:
            pass
    except Exception:
        pass


_install_hooks()

import concourse.bacc as bacc  # noqa: E402
import concourse.mybir as mybir  # noqa: E402
import concourse.tile as tile  # noqa: E402
from concourse.bass_utils import run_bass_kernel_spmd  # noqa: E402
from concourse.kernels.tile_matmul import matmul_tile_kernel  # noqa: E402

_BUILD_CACHE = {}
_LAST_IN_MAPS = {}  # phase key -> in_maps of the most recent kernel() call


def _build_phase_a(m_core):
    key = ("A", m_core)
    if key in _BUILD_CACHE:
        return _BUILD_CACHE[key]
    nc = bacc.Bacc("TRN2", target_bir_lowering=False, debug=False,
                   num_devices=NCORES)
    kxm = nc.dram_tensor("a_ut", [N, m_core], mybir.dt.float8e4,
                         kind="ExternalInput")
    kxn = nc.dram_tensor("adj", [N, N], mybir.dt.float8e4,
                         kind="ExternalInput")
    mxn = nc.dram_tensor("a2", [m_core, N], mybir.dt.bfloat16,
                         kind="ExternalOutput")

    def thresh(nc_, psum, sbuf):
        nc_.vector.tensor_scalar(out=sbuf, in0=psum, scalar1=0.5,
                                 scalar2=None, op0=mybir.AluOpType.is_gt)

    with tile.TileContext(nc) as tc:
        matmul_tile_kernel(tc, kxm.ap(), kxn.ap(), mxn.ap(),
                           psum_evict_fn=thresh)
    nc.compile()
    _BUILD_CACHE[key] = nc
    return nc


def _build_phase_b():
    key = ("B",)
    if key in _BUILD_CACHE:
        return _BUILD_CACHE[key]
    F = mybir.ActivationFunctionType
    nc = bacc.Bacc("TRN2", target_bir_lowering=False, debug=False,
                   num_devices=NCORES)
    bf = mybir.dt.bfloat16
    f32 = mybir.dt.float32
    dr = {}
    for nm in ["ait", "ajt", "a2it", "a2jt"]:
        dr[nm] = nc.dram_tensor(nm, [N, E_CORE], bf, kind="ExternalInput")
    dr["xp"] = nc.dram_tensor("xp", [P, KO, P], bf, kind="ExternalInput")
    dr["xit"] = nc.dram_tensor("xit", [P, E_CORE], bf, kind="ExternalInput")
    dr["xjt"] = nc.dram_tensor("xjt", [P, E_CORE], bf, kind="ExternalInput")
    for br in ["c1", "c2", "c3", "c4"]:
        dr[br + "w1"] = nc.dram_tensor(br + "w1", [P, HID], bf,
                                       kind="ExternalInput")
        dr[br + "w2"] = nc.dram_tensor(br + "w2", [P, 2, HID], bf,
                                       kind="ExternalInput")
        dr[br + "w3"] = nc.dram_tensor(br + "w3", [P, 2, HID], bf,
                                       kind="ExternalInput")
        for b in ["b1", "b2", "b3"]:
            dr[br + b] = nc.dram_tensor(br + b, [P, 2], f32,
                                        kind="ExternalInput")
    dr["ijw1"] = nc.dram_tensor("ijw1", [P, HID], bf, kind="ExternalInput")
    dr["ijw2"] = nc.dram_tensor("ijw2", [P, 2, HID], bf, kind="ExternalInput")
    dr["ijb1"] = nc.dram_tensor("ijb1", [P, 2], f32, kind="ExternalInput")
    dr["ijb2"] = nc.dram_tensor("ijb2", [P, 2], f32, kind="ExternalInput")
    dr["ow1"] = nc.dram_tensor("ow1", [P, 2, HID], bf, kind="ExternalInput")
    dr["ob1"] = nc.dram_tensor("ob1", [P, 2], f32, kind="ExternalInput")
    dr["ow2"] = nc.dram_tensor("ow2", [P, 2, P], bf, kind="ExternalInput")
    dr["ob2"] = nc.dram_tensor("ob2", [1, 1], f32, kind="ExternalInput")
    out = nc.dram_tensor("out", [1, E_CORE], f32, kind="ExternalOutput")

    with tile.TileContext(nc) as tc:
        with (
            tc.tile_pool(name="const", bufs=1) as const,
            tc.tile_pool(name="mask", bufs=3) as maskp,
            tc.tile_pool(name="prod", bufs=3) as prodp,
            tc.tile_pool(name="sres", bufs=1) as sresp,
            tc.tile_pool(name="hid", bufs=6) as hidp,
            tc.tile_pool(name="psum_s", bufs=1, space="PSUM") as psum_s,
            tc.tile_pool(name="psum_m", bufs=4, space="PSUM") as psum_m,
        ):
            x_sb = const.tile([P, KO, P], bf)
            nc.sync.dma_start(x_sb[:], dr["xp"][:])
            w = {}
            for nm in ["c1w1", "c2w1", "c3w1", "c4w1", "ijw1"]:
                w[nm] = const.tile([P, HID], bf, name=nm)
                nc.sync.dma_start(w[nm][:], dr[nm][:])
            for nm in ["c1w2", "c1w3", "c2w2", "c2w3", "c3w2", "c3w3",
                       "c4w2", "c4w3", "ijw2", "ow1", "ow2"]:
                w[nm] = const.tile([P, 2, HID if nm != "ow2" else P], bf,
                                   name=nm)
                nc.sync.dma_start(w[nm][:], dr[nm][:])
            for nm in ["c1b1", "c1b2", "c1b3", "c2b1", "c2b2", "c2b3",
                       "c3b1", "c3b2", "c3b3", "c4b1", "c4b2", "c4b3",
                       "ijb1", "ijb2", "ob1"]:
                w[nm] = const.tile([P, 2], f32, name=nm)
                nc.sync.dma_start(w[nm][:], dr[nm][:])
            w["ob2"] = const.tile([1, 1], f32, name="ob2")
            nc.sync.dma_start(w["ob2"][:], dr["ob2"][:])
            xit_sb = const.tile([P, E_CORE], bf)
            nc.sync.dma_start(xit_sb[:], dr["xit"][:])
            xjt_sb = const.tile([P, E_CORE], bf)
            nc.sync.dma_start(xjt_sb[:], dr["xjt"][:])

            # stage 1: the four masked neighborhood sums, accumulated over
            # 64 node-chunks into 4 persistent PSUM banks
            ps_s = [psum_s.tile([P, E_CORE], f32, name=f"s{b}")
                    for b in range(4)]
            for ko in range(KO):
                mt = {}
                for nm in ["ait", "ajt", "a2it", "a2jt"]:
                    t = maskp.tile([P, E_CORE], bf, tag=nm, name=f"{nm}_{ko}")
                    nc.sync.dma_start(t[:], dr[nm][ko * P:(ko + 1) * P, :])
                    mt[nm] = t
                prods = []
                for b, (u, v) in enumerate([("ait", "ajt"), ("ait", "a2jt"),
                                            ("a2it", "ajt"),
                                            ("a2it", "a2jt")]):
                    pr = prodp.tile([P, E_CORE], bf, tag=f"m{b}",
                                    name=f"m{b}_{ko}")
                    nc.vector.tensor_tensor(pr[:], mt[u][:], mt[v][:],
                                            mybir.AluOpType.mult)
                    prods.append(pr)
                for b in range(4):
                    nc.tensor.matmul(ps_s[b][:], x_sb[:, ko, :], prods[b][:],
                                     start=(ko == 0), stop=(ko == KO - 1))

            s_sb = [sresp.tile([P, E_CORE], bf, name=f"ssb{b}")
                    for b in range(4)]
            for b in range(4):
                nc.vector.tensor_copy(s_sb[b][:], ps_s[b][:])

            def mlp3(src, pre, out_name):
                h1 = hidp.tile([P, 2, E_CORE], bf, tag="h1", name="h1")
                for mo in range(2):
                    ps = psum_m.tile([P, E_CORE], f32, tag="mlp_ps",
                                     name="mlp_ps")
                    nc.tensor.matmul(ps[:], w[pre + "w1"][:, mo * P:(mo + 1) * P],
                                     src[:], start=True, stop=True)
                    nc.scalar.activation(h1[:, mo, :], ps[:], F.Relu,
                                         bias=w[pre + "b1"][:, mo:mo + 1])
                h2 = hidp.tile([P, 2, E_CORE], bf, tag="h2", name="h2")
                for mo in range(2):
                    ps = psum_m.tile([P, E_CORE], f32, tag="mlp_ps",
                                     name="mlp_ps")
                    for ki in range(2):
                        nc.tensor.matmul(
                            ps[:], w[pre + "w2"][:, ki, mo * P:(mo + 1) * P],
                            h1[:, ki, :], start=(ki == 0), stop=(ki == 1))
                    nc.scalar.activation(h2[:, mo, :], ps[:], F.Relu,
                                         bias=w[pre + "b2"][:, mo:mo + 1])
                h3 = hidp.tile([P, 2, E_CORE], bf, tag=out_name, name=out_name)
                for mo in range(2):
                    ps = psum_m.tile([P, E_CORE], f32, tag="mlp_ps",
                                     name="mlp_ps")
                    for ki in range(2):
                        nc.tensor.matmul(
                            ps[:], w[pre + "w3"][:, ki, mo * P:(mo + 1) * P],
                            h2[:, ki, :], start=(ki == 0), stop=(ki == 1))
                    nc.scalar.activation(h3[:, mo, :], ps[:], F.Identity,
                                         bias=w[pre + "b3"][:, mo:mo + 1])
                return h3

            xcn = [mlp3(s_sb[b], pre, f"xcn{b}")
                   for b, pre in enumerate(["c1", "c2", "c3", "c4"])]

            pij = prodp.tile([P, E_CORE], bf, tag="pij")
            nc.vector.tensor_tensor(pij[:], xit_sb[:], xjt_sb[:],
                                    mybir.AluOpType.mult)
            hij = hidp.tile([P, 2, E_CORE], bf, tag="hij")
            for mo in range(2):
                ps = psum_m.tile([P, E_CORE], f32, tag="mlp_ps", name="mlp_ps")
                nc.tensor.matmul(ps[:], w["ijw1"][:, mo * P:(mo + 1) * P],
                                 pij[:], start=True, stop=True)
                nc.scalar.activation(hij[:, mo, :], ps[:], F.Relu,
                                     bias=w["ijb1"][:, mo:mo + 1])
            xij = hidp.tile([P, 2, E_CORE], bf, tag="xij")
            for mo in range(2):
                ps = psum_m.tile([P, E_CORE], f32, tag="mlp_ps", name="mlp_ps")
                for ki in range(2):
                    nc.tensor.matmul(ps[:],
                                     w["ijw2"][:, ki, mo * P:(mo + 1) * P],
                                     hij[:, ki, :], start=(ki == 0),
                                     stop=(ki == 1))
                nc.scalar.activation(xij[:, mo, :], ps[:], F.Identity,
                                     bias=w["ijb2"][:, mo:mo + 1])

            z = hidp.tile([P, 2, E_CORE], bf, tag="z")
            nc.vector.tensor_tensor(z[:], xcn[1][:], xcn[2][:],
                                    mybir.AluOpType.mult)
            nc.vector.tensor_tensor(z[:], z[:], xcn[0][:],
                                    mybir.AluOpType.add)
            nc.vector.tensor_tensor(z[:], z[:], xcn[3][:],
                                    mybir.AluOpType.add)
            nc.vector.tensor_tensor(z[:], z[:], xij[:], mybir.AluOpType.add)

            ho = hidp.tile([P, 2, E_CORE], bf, tag="ho")
            for mo in range(2):
                ps = psum_m.tile([P, E_CORE], f32, tag="mlp_ps", name="mlp_ps")
                for ki in range(2):
                    nc.tensor.matmul(ps[:], w["ow1"][:, ki, mo * P:(mo + 1) * P],
                                     z[:, ki, :], start=(ki == 0),
                                     stop=(ki == 1))
                nc.scalar.activation(ho[:, mo, :], ps[:], F.Relu,
                                     bias=w["ob1"][:, mo:mo + 1])
            ps = psum_m.tile([P, E_CORE], f32, tag="mlp_ps", name="mlp_ps")
            for ki in range(2):
                nc.tensor.matmul(ps[:], w["ow2"][:, ki, :], ho[:, ki, :],
                                 start=(ki == 0), stop=(ki == 1))
            out_sb = sresp.tile([1, E_CORE], f32, name="out_sb")
            nc.scalar.activation(out_sb[:], ps[0:1, :], F.Identity,
                                 bias=w["ob2"][:])
            nc.sync.dma_start(out[:], out_sb[:])

    nc.compile()
    _BUILD_CACHE[key] = nc
    return nc


def _mlp3_weights(pre, params, scale3=None):
    W1, b1, W2, b2, W3, b3 = [np.asarray(t, np.float32) for t in params]
    if scale3 is not None:
        W3 = W3 * scale3
        b3 = b3 * scale3
    out = {}
    out[pre + "w1"] = np.ascontiguousarray(W1.astype(BF16))
    out[pre + "w2"] = np.ascontiguousarray(
        W2.reshape(2, P, HID).transpose(1, 0, 2).astype(BF16))
    out[pre + "w3"] = np.ascontiguousarray(
        W3.reshape(2, P, HID).transpose(1, 0, 2).astype(BF16))
    out[pre + "b1"] = np.ascontiguousarray(b1.reshape(2, P).T.astype(np.float32))
    out[pre + "b2"] = np.ascontiguousarray(b2.reshape(2, P).T.astype(np.float32))
    out[pre + "b3"] = np.ascontiguousarray(b3.reshape(2, P).T.astype(np.float32))
    return out


def kernel(x, adj, tar_ei, alpha, beta, p_cn1, p_cn2, p_cn4, p_ij, p_out):
    x = np.asarray(x, np.float32)
    adj = np.asarray(adj, np.float32)
    tar = np.asarray(tar_ei).astype(np.int64)
    assert x.shape == (N, IN_CH) and adj.shape == (N, N)
    assert tar.shape == (2, E)
    cores = list(range(NCORES))

    # ---------------- phase A: adj2 rows for unique endpoints ----------------
    uniq, inv = np.unique(tar, return_inverse=True)
    inv = inv.reshape(tar.shape)
    U = uniq.size
    m_core = 768 if U <= 768 * NCORES else 1024
    m_total = m_core * NCORES

    adj_u8 = (adj != 0).astype(np.uint8)
    adj_fp8 = (adj_u8 * ONE_FP8).view(FP8)
    a_ut = np.zeros((N, m_total), np.uint8)
    a_ut[:, :U] = adj_u8[uniq].T
    a_ut_fp8 = (a_ut * ONE_FP8).view(FP8)

    nc_a = _build_phase_a(m_core)
    in_maps_a = [
        {"a_ut": np.ascontiguousarray(a_ut_fp8[:, c * m_core:(c + 1) * m_core]),
         "adj": adj_fp8}
        for c in cores
    ]
    _LAST_IN_MAPS[("A", m_core)] = in_maps_a
    res_a = run_bass_kernel_spmd(nc_a, in_maps_a, cores)
    a2_rows = np.concatenate([res_a.results[c]["a2"] for c in cores], axis=0)
    a2_rows = a2_rows[:U]  # [U, N] bf16, exact 0.0/1.0

    # ---------------- host: per-edge gathers (sharding) ----------------------
    i_all, j_all = tar[0], tar[1]
    inv_i, inv_j = inv[0], inv[1]
    adj_bf = adj_u8.astype(BF16)
    x_bf = x.astype(BF16)
    xp = np.ascontiguousarray(x_bf.reshape(KO, P, P).transpose(1, 0, 2))

    alpha = np.asarray(alpha, np.float64).reshape(3)
    beta_v = float(np.asarray(beta, np.float64).reshape(1)[0])
    a = np.cumprod(1.0 / (1.0 + np.exp(-alpha))).astype(np.float32)

    wmaps = {}
    wmaps.update(_mlp3_weights("c1", p_cn1, a[0]))
    wmaps.update(_mlp3_weights("c2", p_cn2, a[1]))
    wmaps.update(_mlp3_weights("c3", p_cn2, None))
    wmaps.update(_mlp3_weights("c4", p_cn4, a[2]))
    Wij1, bij1, Wij2, bij2 = [np.asarray(t, np.float32) for t in p_ij]
    Wij2 = Wij2 * beta_v
    bij2 = bij2 * beta_v
    wmaps["ijw1"] = np.ascontiguousarray(Wij1.astype(BF16))
    wmaps["ijw2"] = np.ascontiguousarray(
        Wij2.reshape(2, P, HID).transpose(1, 0, 2).astype(BF16))
    wmaps["ijb1"] = np.ascontiguousarray(bij1.reshape(2, P).T.astype(np.float32))
    wmaps["ijb2"] = np.ascontiguousarray(bij2.reshape(2, P).T.astype(np.float32))
    Wo1, bo1, Wo2, bo2 = [np.asarray(t, np.float32) for t in p_out]
    wmaps["ow1"] = np.ascontiguousarray(
        Wo1.reshape(2, P, HID).transpose(1, 0, 2).astype(BF16))
    wmaps["ob1"] = np.ascontiguousarray(bo1.reshape(2, P).T.astype(np.float32))
    ow2 = np.zeros((2, P, P), np.float32)
    ow2[:, :, 0] = Wo2.reshape(2, P)
    wmaps["ow2"] = np.ascontiguousarray(ow2.transpose(1, 0, 2).astype(BF16))
    wmaps["ob2"] = np.full((1, 1), np.float32(bo2.reshape(1)[0]), np.float32)

    in_maps_b = []
    for c in cores:
        sl = slice(c * E_CORE, (c + 1) * E_CORE)
        i_s, j_s = i_all[sl], j_all[sl]
        m = dict(wmaps)
        m["ait"] = np.ascontiguousarray(adj_bf[i_s].T)
        m["ajt"] = np.ascontiguousarray(adj_bf[j_s].T)
        m["a2it"] = np.ascontiguousarray(a2_rows[inv_i[sl]].T)
        m["a2jt"] = np.ascontiguousarray(a2_rows[inv_j[sl]].T)
        m["xp"] = xp
        m["xit"] = np.ascontiguousarray(x_bf[i_s].T)
        m["xjt"] = np.ascontiguousarray(x_bf[j_s].T)
        in_maps_b.append(m)

    nc_b = _build_phase_b()
    _LAST_IN_MAPS[("B",)] = in_maps_b
    res_b = run_bass_kernel_spmd(nc_b, in_maps_b, cores)
    out = np.concatenate(
        [np.asarray(res_b.results[c]["out"][0], np.float32) for c in cores])
    return out[:, None]


# revision 11
# speedup vs baseline: 3.2620x; 3.2620x over previous
"""CN2LinkPredictor kernel for 8 Trainium2 NeuronCores.

Strategy:
  Phase A (device): A2 = (A_u @ adj) > 0 for the unique edge-endpoint rows
    A_u of adj, sharded across 8 cores, fp8 matmul (exact: 0/1 inputs with
    fp32 PSUM accumulation) with a >0.5 threshold on PSUM eviction.
  Host: gather/transpose per-edge mask rows (sharding/data-distribution only).
  Phase B (device): per-edge masked neighborhood sums (4 mask products on
    DVE, matmuls against x on PE) + all MLPs, edges sharded across 8 cores,
    computed in transposed layout [channels, edges].

All FLOPs (matmuls, mask intersections, MLPs, thresholds) run on device.
Host does sharding, gathers, transposes, dtype casts and weight scaling
(folding the cumprod(sigmoid(alpha)) / beta scalars into the last-layer
weights of each branch).
"""
import sys
import types

import numpy as np
import ml_dtypes

BF16 = ml_dtypes.bfloat16
FP8 = ml_dtypes.float8_e4m3
ONE_FP8 = np.uint8(0x38)  # bit pattern of 1.0 in float8_e4m3

N = 8192
IN_CH = 128
HID = 256
E = 4096
NCORES = 8
E_CORE = E // NCORES
P = 128
KO = N // P


def _install_hooks():
    """Make antenv.axon_hooks importable so traced runs work (optional)."""
    if "antenv.axon_hooks" in sys.modules:
        return
    try:
        import antenv
        mod = types.ModuleType("antenv.axon_hooks")
        mod._hook = None

        def set_axon_ntff_profile_hook(h):
            mod._hook = h

        def get_axon_ntff_profile_hook():
            return mod._hook

        mod.set_axon_ntff_profile_hook = set_axon_ntff_profile_hook
        mod.get_axon_ntff_profile_hook = get_axon_ntff_profile_hook
        sys.modules["antenv.axon_hooks"] = mod
        antenv.axon_hooks = mod
        try:
            from trn_agent_boot.trn_boot import _ntff_profile_via_ctypes
            hook = _ntff_profile_via_ctypes("/opt/axon/libaxon_pjrt.so")
            if hook is not None:
                set_axon_ntff_profile_hook(hook)
        except Exception:
            pass
    except Exception:
        pass


_install_hooks()

import concourse.bacc as bacc  # noqa: E402
import concourse.mybir as mybir  # noqa: E402
import concourse.tile as tile  # noqa: E402
from concourse.bass_utils import run_bass_kernel_spmd  # noqa: E402
from concourse.kernels.tile_matmul import matmul_tile_kernel  # noqa: E402

_BUILD_CACHE = {}
_LAST_IN_MAPS = {}  # phase key -> in_maps of the most recent kernel() call


W32 = 256  # packed adjacency row width in uint32 (8192 bits)
ACH = 16   # gather chunk (steps per DMA transfer) for phase A


def _build_phase_a2(NB, steps):
    """OR-reduce pre-gathered bit-packed neighbor rows: A2[u] = OR adj[v]."""
    key = ("A2", NB, tuple(steps))
    if key in _BUILD_CACHE:
        return _BUILD_CACHE[key], key
    nc = bacc.Bacc("TRN2", target_bir_lowering=False, debug=False,
                   num_devices=NCORES)
    S_TOT = int(sum(steps))
    gath = nc.dram_tensor("gath", [S_TOT, P, W32], mybir.dt.uint32,
                          kind="ExternalInput")
    outp = nc.dram_tensor("a2p", [NB * P, W32], mybir.dt.uint32,
                          kind="ExternalOutput")
    u32 = mybir.dt.uint32
    OR = mybir.AluOpType.bitwise_or
    with tile.TileContext(nc) as tc:
        with (
            tc.tile_pool(name="load", bufs=4) as loadp,
            tc.tile_pool(name="accp", bufs=2) as accp,
        ):
            srow = 0
            qi = 0
            for b in range(NB):
                acc = accp.tile([P, W32], u32, tag="acc", name=f"acc{b}")
                done = 0
                first = True
                while done < steps[b]:
                    cs = min(ACH, steps[b] - done)
                    t = loadp.tile([P, ACH, W32], u32, tag="g",
                                   name=f"g{b}_{done}")
                    eng = nc.sync if qi % 2 == 0 else nc.scalar
                    qi += 1
                    eng.dma_start(
                        t[:, :cs, :],
                        gath[srow:srow + cs].rearrange("s p w -> p s w"))
                    srow += cs
                    done += cs
                    width = cs
                    while width > 1:
                        h = width // 2
                        nc.vector.tensor_tensor(t[:, :h, :], t[:, :h, :],
                                                t[:, width - h:width, :], OR)
                        width -= h
                    if first:
                        nc.vector.tensor_copy(acc[:], t[:, 0, :])
                        first = False
                    else:
                        nc.vector.tensor_tensor(acc[:], acc[:], t[:, 0, :],
                                                OR)
                nc.sync.dma_start(outp[b * P:(b + 1) * P, :], acc[:])
    nc.compile()
    _BUILD_CACHE[key] = nc
    return nc, key


def _build_phase_b(LCH=8, PCH=4, tail_dve=False, dual_q=True, mbufs=2, pbufs=3):
    key = ("B", LCH, PCH, tail_dve, dual_q, mbufs, pbufs)
    if key in _BUILD_CACHE:
        return _BUILD_CACHE[key]
    F = mybir.ActivationFunctionType
    nc = bacc.Bacc("TRN2", target_bir_lowering=False, debug=False,
                   num_devices=NCORES)
    bf = mybir.dt.bfloat16
    f32 = mybir.dt.float32
    dr = {}
    for nm in ["ait", "ajt", "a2it", "a2jt"]:
        dr[nm] = nc.dram_tensor(nm, [N, E_CORE], bf, kind="ExternalInput")
    dr["xp"] = nc.dram_tensor("xp", [P, KO, P], bf, kind="ExternalInput")
    dr["xit"] = nc.dram_tensor("xit", [P, E_CORE], bf, kind="ExternalInput")
    dr["xjt"] = nc.dram_tensor("xjt", [P, E_CORE], bf, kind="ExternalInput")
    for br in ["c1", "c2", "c3", "c4"]:
        dr[br + "w1"] = nc.dram_tensor(br + "w1", [P, HID], bf,
                                       kind="ExternalInput")
        dr[br + "w2"] = nc.dram_tensor(br + "w2", [P, 2, HID], bf,
                                       kind="ExternalInput")
        dr[br + "w3"] = nc.dram_tensor(br + "w3", [P, 2, HID], bf,
                                       kind="ExternalInput")
        for b in ["b1", "b2", "b3"]:
            dr[br + b] = nc.dram_tensor(br + b, [P, 2], f32,
                                        kind="ExternalInput")
    dr["ijw1"] = nc.dram_tensor("ijw1", [P, HID], bf, kind="ExternalInput")
    dr["ijw2"] = nc.dram_tensor("ijw2", [P, 2, HID], bf, kind="ExternalInput")
    dr["ijb1"] = nc.dram_tensor("ijb1", [P, 2], f32, kind="ExternalInput")
    dr["ijb2"] = nc.dram_tensor("ijb2", [P, 2], f32, kind="ExternalInput")
    dr["ow1"] = nc.dram_tensor("ow1", [P, 2, HID], bf, kind="ExternalInput")
    dr["ob1"] = nc.dram_tensor("ob1", [P, 2], f32, kind="ExternalInput")
    dr["ow2"] = nc.dram_tensor("ow2", [P, 2, P], bf, kind="ExternalInput")
    dr["ob2"] = nc.dram_tensor("ob2", [1, 1], f32, kind="ExternalInput")
    out = nc.dram_tensor("out", [1, E_CORE], f32, kind="ExternalOutput")

    with tile.TileContext(nc) as tc:
        with (
            tc.tile_pool(name="const", bufs=1) as const,
            tc.tile_pool(name="mask", bufs=mbufs) as maskp,
            tc.tile_pool(name="prod", bufs=pbufs) as prodp,
            tc.tile_pool(name="sres", bufs=1) as sresp,
            tc.tile_pool(name="hid", bufs=2) as hidp,
            tc.tile_pool(name="psum_s", bufs=1, space="PSUM") as psum_s,
            tc.tile_pool(name="psum_m", bufs=4, space="PSUM") as psum_m,
        ):
            # one-time loads ride the scalar queue (masks own sync+gpsimd)
            x_sb = const.tile([P, KO, P], bf)
            nc.scalar.dma_start(x_sb[:], dr["xp"][:])
            w = {}
            for nm in ["c1w1", "c2w1", "c3w1", "c4w1", "ijw1"]:
                w[nm] = const.tile([P, HID], bf, name=nm)
                nc.scalar.dma_start(w[nm][:], dr[nm][:])
            for nm in ["c1w2", "c1w3", "c2w2", "c2w3", "c3w2", "c3w3",
                       "c4w2", "c4w3", "ijw2", "ow1", "ow2"]:
                w[nm] = const.tile([P, 2, HID if nm != "ow2" else P], bf,
                                   name=nm)
                nc.scalar.dma_start(w[nm][:], dr[nm][:])
            for nm in ["c1b1", "c1b2", "c1b3", "c2b1", "c2b2", "c2b3",
                       "c3b1", "c3b2", "c3b3", "c4b1", "c4b2", "c4b3",
                       "ijb1", "ijb2", "ob1"]:
                w[nm] = const.tile([P, 2], f32, name=nm)
                nc.scalar.dma_start(w[nm][:], dr[nm][:])
            w["ob2"] = const.tile([1, 1], f32, name="ob2")
            nc.scalar.dma_start(w["ob2"][:], dr["ob2"][:])
            xit_sb = const.tile([P, E_CORE], bf)
            nc.scalar.dma_start(xit_sb[:], dr["xit"][:])
            xjt_sb = const.tile([P, E_CORE], bf)
            nc.scalar.dma_start(xjt_sb[:], dr["xjt"][:])

            # stage 1: the four masked neighborhood sums. Mask chunks of 8
            # node-blocks (1 MiB transfers) alternate between the sync and
            # gpsimd DMA queues so transfers run on two queues in parallel.
            ps_s = [psum_s.tile([P, E_CORE], f32, name=f"s{b}")
                    for b in range(4)]
            for kl in range(KO // LCH):
                dma_eng = nc.sync if (kl % 2 == 0 or not dual_q) else nc.gpsimd
                mt = {}
                for nm in ["ait", "ajt", "a2it", "a2jt"]:
                    t = maskp.tile([P, LCH, E_CORE], bf, tag=nm,
                                   name=f"{nm}_{kl}")
                    src = dr[nm][kl * LCH * P:(kl + 1) * LCH * P, :]
                    dma_eng.dma_start(
                        t[:], src.rearrange("(c p) e -> p c e", p=P))
                    mt[nm] = t
                for pc in range(LCH // PCH):
                    sl = slice(pc * PCH, (pc + 1) * PCH)
                    prods = []
                    for b, (u, v) in enumerate([("ait", "ajt"),
                                                ("ait", "a2jt"),
                                                ("a2it", "ajt"),
                                                ("a2it", "a2jt")]):
                        pr = prodp.tile([P, PCH, E_CORE], bf, tag=f"m{b}",
                                        name=f"m{b}_{kl}_{pc}")
                        nc.vector.tensor_tensor(pr[:], mt[u][:, sl, :],
                                                mt[v][:, sl, :],
                                                mybir.AluOpType.mult)
                        prods.append(pr)
                    for c in range(PCH):
                        ko = kl * LCH + pc * PCH + c
                        for b in range(4):
                            nc.tensor.matmul(ps_s[b][:], x_sb[:, ko, :],
                                             prods[b][:, c, :],
                                             start=(ko == 0),
                                             stop=(ko == KO - 1))

            s_sb = [sresp.tile([P, E_CORE], bf, name=f"ssb{b}")
                    for b in range(4)]
            for b in range(4):
                nc.scalar.activation(s_sb[b][:], ps_s[b][:], F.Copy, bias=0.0)

            def act_relu(dst, ps, bias_ap, mo):
                # alternate ACT / DVE so the two engines split the MLP tail
                if mo == 0 or not tail_dve:
                    nc.scalar.activation(dst, ps, F.Relu, bias=bias_ap)
                else:
                    nc.vector.tensor_scalar(out=dst, in0=ps, scalar1=bias_ap,
                                            scalar2=0.0,
                                            op0=mybir.AluOpType.add,
                                            op1=mybir.AluOpType.max)

            def act_ident(dst, ps, bias_ap, mo):
                if mo == 0 or not tail_dve:
                    nc.scalar.activation(dst, ps, F.Identity, bias=bias_ap)
                else:
                    nc.vector.tensor_scalar(out=dst, in0=ps, scalar1=bias_ap,
                                            scalar2=None,
                                            op0=mybir.AluOpType.add)

            def mlp3(src, pre, out_name):
                h1 = hidp.tile([P, 2, E_CORE], bf, tag="h1", name="h1")
                for mo in range(2):
                    ps = psum_m.tile([P, E_CORE], f32, tag="mlp_ps",
                                     name="mlp_ps")
                    nc.tensor.matmul(ps[:],
                                     w[pre + "w1"][:, mo * P:(mo + 1) * P],
                                     src[:], start=True, stop=True)
                    act_relu(h1[:, mo, :], ps[:], w[pre + "b1"][:, mo:mo + 1],
                             mo)
                h2 = hidp.tile([P, 2, E_CORE], bf, tag="h2", name="h2")
                for mo in range(2):
                    ps = psum_m.tile([P, E_CORE], f32, tag="mlp_ps",
                                     name="mlp_ps")
                    for ki in range(2):
                        nc.tensor.matmul(
                            ps[:], w[pre + "w2"][:, ki, mo * P:(mo + 1) * P],
                            h1[:, ki, :], start=(ki == 0), stop=(ki == 1))
                    act_relu(h2[:, mo, :], ps[:], w[pre + "b2"][:, mo:mo + 1],
                             mo)
                h3 = hidp.tile([P, 2, E_CORE], bf, tag=out_name, name=out_name)
                for mo in range(2):
                    ps = psum_m.tile([P, E_CORE], f32, tag="mlp_ps",
                                     name="mlp_ps")
                    for ki in range(2):
                        nc.tensor.matmul(
                            ps[:], w[pre + "w3"][:, ki, mo * P:(mo + 1) * P],
                            h2[:, ki, :], start=(ki == 0), stop=(ki == 1))
                    act_ident(h3[:, mo, :], ps[:], w[pre + "b3"][:, mo:mo + 1],
                              mo)
                return h3

            xcn = [mlp3(s_sb[b], pre, f"xcn{b}")
                   for b, pre in enumerate(["c1", "c2", "c3", "c4"])]

            pij = prodp.tile([P, E_CORE], bf, tag="pij")
            nc.vector.tensor_tensor(pij[:], xit_sb[:], xjt_sb[:],
                                    mybir.AluOpType.mult)
            hij = hidp.tile([P, 2, E_CORE], bf, tag="hij")
            for mo in range(2):
                ps = psum_m.tile([P, E_CORE], f32, tag="mlp_ps", name="mlp_ps")
                nc.tensor.matmul(ps[:], w["ijw1"][:, mo * P:(mo + 1) * P],
                                 pij[:], start=True, stop=True)
                act_relu(hij[:, mo, :], ps[:], w["ijb1"][:, mo:mo + 1], mo)
            xij = hidp.tile([P, 2, E_CORE], bf, tag="xij")
            for mo in range(2):
                ps = psum_m.tile([P, E_CORE], f32, tag="mlp_ps", name="mlp_ps")
                for ki in range(2):
                    nc.tensor.matmul(ps[:],
                                     w["ijw2"][:, ki, mo * P:(mo + 1) * P],
                                     hij[:, ki, :], start=(ki == 0),
                                     stop=(ki == 1))
                act_ident(xij[:, mo, :], ps[:], w["ijb2"][:, mo:mo + 1], mo)

            z = hidp.tile([P, 2, E_CORE], bf, tag="z")
            nc.vector.tensor_tensor(z[:], xcn[1][:], xcn[2][:],
                                    mybir.AluOpType.mult)
            nc.vector.tensor_tensor(z[:], z[:], xcn[0][:],
                                    mybir.AluOpType.add)
            nc.vector.tensor_tensor(z[:], z[:], xcn[3][:],
                                    mybir.AluOpType.add)
            nc.vector.tensor_tensor(z[:], z[:], xij[:], mybir.AluOpType.add)

            ho = hidp.tile([P, 2, E_CORE], bf, tag="ho")
            for mo in range(2):
                ps = psum_m.tile([P, E_CORE], f32, tag="mlp_ps", name="mlp_ps")
                for ki in range(2):
                    nc.tensor.matmul(ps[:], w["ow1"][:, ki, mo * P:(mo + 1) * P],
                                     z[:, ki, :], start=(ki == 0),
                                     stop=(ki == 1))
                act_relu(ho[:, mo, :], ps[:], w["ob1"][:, mo:mo + 1], mo)
            ps = psum_m.tile([P, E_CORE], f32, tag="mlp_ps", name="mlp_ps")
            for ki in range(2):
                nc.tensor.matmul(ps[:], w["ow2"][:, ki, :], ho[:, ki, :],
                                 start=(ki == 0), stop=(ki == 1))
            out_sb = sresp.tile([1, E_CORE], f32, name="out_sb")
            nc.scalar.activation(out_sb[:], ps[0:1, :], F.Identity,
                                 bias=w["ob2"][:])
            nc.sync.dma_start(out[:], out_sb[:])

    nc.compile()
    _BUILD_CACHE[key] = nc
    return nc


def _mlp3_weights(pre, params, scale3=None):
    W1, b1, W2, b2, W3, b3 = [np.asarray(t, np.float32) for t in params]
    if scale3 is not None:
        W3 = W3 * scale3
        b3 = b3 * scale3
    out = {}
    out[pre + "w1"] = np.ascontiguousarray(W1.astype(BF16))
    out[pre + "w2"] = np.ascontiguousarray(
        W2.reshape(2, P, HID).transpose(1, 0, 2).astype(BF16))
    out[pre + "w3"] = np.ascontiguousarray(
        W3.reshape(2, P, HID).transpose(1, 0, 2).astype(BF16))
    out[pre + "b1"] = np.ascontiguousarray(b1.reshape(2, P).T.astype(np.float32))
    out[pre + "b2"] = np.ascontiguousarray(b2.reshape(2, P).T.astype(np.float32))
    out[pre + "b3"] = np.ascontiguousarray(b3.reshape(2, P).T.astype(np.float32))
    return out


def kernel(x, adj, tar_ei, alpha, beta, p_cn1, p_cn2, p_cn4, p_ij, p_out):
    x = np.asarray(x, np.float32)
    adj = np.asarray(adj, np.float32)
    tar = np.asarray(tar_ei).astype(np.int64)
    assert x.shape == (N, IN_CH) and adj.shape == (N, N)
    assert tar.shape == (2, E)
    cores = list(range(NCORES))

    # ---------------- phase A: adj2 rows for unique endpoints ----------------
    # A2[u] = OR over v in nbr(u) of adj[v], on bit-packed rows. The host
    # gathers the needed packed rows per shard (data distribution); the
    # device does the OR reduction.
    uniq, inv = np.unique(tar, return_inverse=True)
    inv = inv.reshape(tar.shape)
    U = uniq.size
    NB = int(np.ceil(U / (NCORES * P)))
    U_pad = NB * P * NCORES

    adj_u8 = (adj != 0).astype(np.uint8)
    adjP = np.packbits(adj_u8, axis=1)
    adjP = np.vstack([adjP, np.zeros((1, N // 8), np.uint8)])  # zero row at N
    adjP32 = np.ascontiguousarray(adjP).view(np.uint32)

    sub = adj_u8[uniq]
    deg = sub.sum(1).astype(np.int64)
    degp = np.concatenate([deg, np.zeros(U_pad - U, np.int64)])
    order = np.argsort(degp, kind="stable")
    assign = order.reshape(NB * P, NCORES).T  # [core, pos] -> padded slot

    rows, cols = np.nonzero(sub)
    maxdeg = max(int(deg.max()), 1)
    starts = np.zeros(U + 1, np.int64)
    np.cumsum(deg, out=starts[1:])
    rank = np.arange(len(cols)) - starts[rows]
    nbrmat = np.full((U_pad, maxdeg), N, np.int64)
    nbrmat[rows, rank] = cols

    degp_sorted = degp[order]
    steps = tuple(
        max(int(degp_sorted[(b + 1) * P * NCORES - 1]), 1) for b in range(NB))

    in_maps_a = []
    for c in cores:
        blocks = []
        for b in range(NB):
            nodes = assign[c, b * P:(b + 1) * P]
            blocks.append(nbrmat[nodes, :steps[b]].T)
        I = np.concatenate(blocks, axis=0)
        in_maps_a.append({"gath": np.ascontiguousarray(adjP32[I])})

    nc_a, akey = _build_phase_a2(NB, steps)
    _LAST_IN_MAPS.clear()
    _LAST_IN_MAPS[akey] = in_maps_a
    res_a = run_bass_kernel_spmd(nc_a, in_maps_a, cores)

    a2_pad = np.empty((U_pad, W32), np.uint32)
    for c in cores:
        a2_pad[assign[c]] = res_a.results[c]["a2p"]
    a2_rows = np.unpackbits(a2_pad[:U].view(np.uint8), axis=1).astype(BF16)

    # ---------------- host: per-edge gathers (sharding) ----------------------
    i_all, j_all = tar[0], tar[1]
    inv_i, inv_j = inv[0], inv[1]
    adj_bf = adj_u8.astype(BF16)
    x_bf = x.astype(BF16)
    xp = np.ascontiguousarray(x_bf.reshape(KO, P, P).transpose(1, 0, 2))

    alpha = np.asarray(alpha, np.float64).reshape(3)
    beta_v = float(np.asarray(beta, np.float64).reshape(1)[0])
    a = np.cumprod(1.0 / (1.0 + np.exp(-alpha))).astype(np.float32)

    wmaps = {}
    wmaps.update(_mlp3_weights("c1", p_cn1, a[0]))
    wmaps.update(_mlp3_weights("c2", p_cn2, a[1]))
    wmaps.update(_mlp3_weights("c3", p_cn2, None))
    wmaps.update(_mlp3_weights("c4", p_cn4, a[2]))
    Wij1, bij1, Wij2, bij2 = [np.asarray(t, np.float32) for t in p_ij]
    Wij2 = Wij2 * beta_v
    bij2 = bij2 * beta_v
    wmaps["ijw1"] = np.ascontiguousarray(Wij1.astype(BF16))
    wmaps["ijw2"] = np.ascontiguousarray(
        Wij2.reshape(2, P, HID).transpose(1, 0, 2).astype(BF16))
    wmaps["ijb1"] = np.ascontiguousarray(bij1.reshape(2, P).T.astype(np.float32))
    wmaps["ijb2"] = np.ascontiguousarray(bij2.reshape(2, P).T.astype(np.float32))
    Wo1, bo1, Wo2, bo2 = [np.asarray(t, np.float32) for t in p_out]
    wmaps["ow1"] = np.ascontiguousarray(
        Wo1.reshape(2, P, HID).transpose(1, 0, 2).astype(BF16))
    wmaps["ob1"] = np.ascontiguousarray(bo1.reshape(2, P).T.astype(np.float32))
    ow2 = np.zeros((2, P, P), np.float32)
    ow2[:, :, 0] = Wo2.reshape(2, P)
    wmaps["ow2"] = np.ascontiguousarray(ow2.transpose(1, 0, 2).astype(BF16))
    wmaps["ob2"] = np.full((1, 1), np.float32(bo2.reshape(1)[0]), np.float32)

    in_maps_b = []
    for c in cores:
        sl = slice(c * E_CORE, (c + 1) * E_CORE)
        i_s, j_s = i_all[sl], j_all[sl]
        m = dict(wmaps)
        m["ait"] = np.ascontiguousarray(adj_bf[i_s].T)
        m["ajt"] = np.ascontiguousarray(adj_bf[j_s].T)
        m["a2it"] = np.ascontiguousarray(a2_rows[inv_i[sl]].T)
        m["a2jt"] = np.ascontiguousarray(a2_rows[inv_j[sl]].T)
        m["xp"] = xp
        m["xit"] = np.ascontiguousarray(x_bf[i_s].T)
        m["xjt"] = np.ascontiguousarray(x_bf[j_s].T)
        in_maps_b.append(m)

    nc_b = _build_phase_b()
    _LAST_IN_MAPS[("B", 8, 4, False, True, 2, 3)] = in_maps_b
    res_b = run_bass_kernel_spmd(nc_b, in_maps_b, cores)
    out = np.concatenate(
        [np.asarray(res_b.results[c]["out"][0], np.float32) for c in cores])
    return out[:, None]


# revision 12
# speedup vs baseline: 3.4827x; 1.0677x over previous
"""CN2LinkPredictor kernel for 8 Trainium2 NeuronCores.

Strategy:
  Phase A (device): A2 = (A_u @ adj) > 0 for the unique edge-endpoint rows
    A_u of adj, sharded across 8 cores, fp8 matmul (exact: 0/1 inputs with
    fp32 PSUM accumulation) with a >0.5 threshold on PSUM eviction.
  Host: gather/transpose per-edge mask rows (sharding/data-distribution only).
  Phase B (device): per-edge masked neighborhood sums (4 mask products on
    DVE, matmuls against x on PE) + all MLPs, edges sharded across 8 cores,
    computed in transposed layout [channels, edges].

All FLOPs (matmuls, mask intersections, MLPs, thresholds) run on device.
Host does sharding, gathers, transposes, dtype casts and weight scaling
(folding the cumprod(sigmoid(alpha)) / beta scalars into the last-layer
weights of each branch).
"""
import sys
import types

import numpy as np
import ml_dtypes

BF16 = ml_dtypes.bfloat16
FP8 = ml_dtypes.float8_e4m3
ONE_FP8 = np.uint8(0x38)  # bit pattern of 1.0 in float8_e4m3

N = 8192
IN_CH = 128
HID = 256
E = 4096
NCORES = 8
E_CORE = E // NCORES
P = 128
KO = N // P


def _install_hooks():
    """Make antenv.axon_hooks importable so traced runs work (optional)."""
    if "antenv.axon_hooks" in sys.modules:
        return
    try:
        import antenv
        mod = types.ModuleType("antenv.axon_hooks")
        mod._hook = None

        def set_axon_ntff_profile_hook(h):
            mod._hook = h

        def get_axon_ntff_profile_hook():
            return mod._hook

        mod.set_axon_ntff_profile_hook = set_axon_ntff_profile_hook
        mod.get_axon_ntff_profile_hook = get_axon_ntff_profile_hook
        sys.modules["antenv.axon_hooks"] = mod
        antenv.axon_hooks = mod
        try:
            from trn_agent_boot.trn_boot import _ntff_profile_via_ctypes
            hook = _ntff_profile_via_ctypes("/opt/axon/libaxon_pjrt.so")
            if hook is not None:
                set_axon_ntff_profile_hook(hook)
        except Exception:
            pass
    except Exception:
        pass


_install_hooks()

import concourse.bacc as bacc  # noqa: E402
import concourse.mybir as mybir  # noqa: E402
import concourse.tile as tile  # noqa: E402
from concourse.bass_utils import run_bass_kernel_spmd  # noqa: E402
from concourse.kernels.tile_matmul import matmul_tile_kernel  # noqa: E402

_BUILD_CACHE = {}
_LAST_IN_MAPS = {}  # phase key -> in_maps of the most recent kernel() call


W32 = 256  # packed adjacency row width in uint32 (8192 bits)
ACH = 16   # gather chunk (steps per DMA transfer) for phase A


def _build_phase_a2(NB, steps):
    """OR-reduce pre-gathered bit-packed neighbor rows: A2[u] = OR adj[v]."""
    key = ("A2", NB, tuple(steps))
    if key in _BUILD_CACHE:
        return _BUILD_CACHE[key], key
    nc = bacc.Bacc("TRN2", target_bir_lowering=False, debug=False,
                   num_devices=NCORES)
    S_TOT = int(sum(steps))
    gath = nc.dram_tensor("gath", [S_TOT, P, W32], mybir.dt.uint32,
                          kind="ExternalInput")
    outp = nc.dram_tensor("a2p", [NB * P, W32], mybir.dt.uint32,
                          kind="ExternalOutput")
    u32 = mybir.dt.uint32
    OR = mybir.AluOpType.bitwise_or
    with tile.TileContext(nc) as tc:
        with (
            tc.tile_pool(name="load", bufs=4) as loadp,
            tc.tile_pool(name="accp", bufs=2) as accp,
        ):
            srow = 0
            qi = 0
            for b in range(NB):
                acc = accp.tile([P, W32], u32, tag="acc", name=f"acc{b}")
                done = 0
                first = True
                while done < steps[b]:
                    cs = min(ACH, steps[b] - done)
                    t = loadp.tile([P, ACH, W32], u32, tag="g",
                                   name=f"g{b}_{done}")
                    eng = nc.sync if qi % 2 == 0 else nc.scalar
                    qi += 1
                    eng.dma_start(
                        t[:, :cs, :],
                        gath[srow:srow + cs].rearrange("s p w -> p s w"))
                    srow += cs
                    done += cs
                    width = cs
                    while width > 1:
                        h = width // 2
                        nc.vector.tensor_tensor(t[:, :h, :], t[:, :h, :],
                                                t[:, width - h:width, :], OR)
                        width -= h
                    if first:
                        nc.vector.tensor_copy(acc[:], t[:, 0, :])
                        first = False
                    else:
                        nc.vector.tensor_tensor(acc[:], acc[:], t[:, 0, :],
                                                OR)
                nc.sync.dma_start(outp[b * P:(b + 1) * P, :], acc[:])
    nc.compile()
    _BUILD_CACHE[key] = nc
    return nc, key


def _build_phase_b(LCH=4, PCH=4, tail_dve=False, dual_q=False, mbufs=3, pbufs=3):
    key = ("B", LCH, PCH, tail_dve, dual_q, mbufs, pbufs)
    if key in _BUILD_CACHE:
        return _BUILD_CACHE[key], key
    F = mybir.ActivationFunctionType
    nc = bacc.Bacc("TRN2", target_bir_lowering=False, debug=False,
                   num_devices=NCORES)
    bf = mybir.dt.bfloat16
    f32 = mybir.dt.float32
    dr = {}
    for nm in ["ait", "ajt", "a2it", "a2jt"]:
        dr[nm] = nc.dram_tensor(nm, [N, E_CORE], bf, kind="ExternalInput")
    dr["xp"] = nc.dram_tensor("xp", [P, KO, P], bf, kind="ExternalInput")
    dr["xit"] = nc.dram_tensor("xit", [P, E_CORE], bf, kind="ExternalInput")
    dr["xjt"] = nc.dram_tensor("xjt", [P, E_CORE], bf, kind="ExternalInput")
    for br in ["c1", "c2", "c3", "c4"]:
        dr[br + "w1"] = nc.dram_tensor(br + "w1", [P, HID], bf,
                                       kind="ExternalInput")
        dr[br + "w2"] = nc.dram_tensor(br + "w2", [P, 2, HID], bf,
                                       kind="ExternalInput")
        dr[br + "w3"] = nc.dram_tensor(br + "w3", [P, 2, HID], bf,
                                       kind="ExternalInput")
        for b in ["b1", "b2", "b3"]:
            dr[br + b] = nc.dram_tensor(br + b, [P, 2], f32,
                                        kind="ExternalInput")
    dr["ijw1"] = nc.dram_tensor("ijw1", [P, HID], bf, kind="ExternalInput")
    dr["ijw2"] = nc.dram_tensor("ijw2", [P, 2, HID], bf, kind="ExternalInput")
    dr["ijb1"] = nc.dram_tensor("ijb1", [P, 2], f32, kind="ExternalInput")
    dr["ijb2"] = nc.dram_tensor("ijb2", [P, 2], f32, kind="ExternalInput")
    dr["ow1"] = nc.dram_tensor("ow1", [P, 2, HID], bf, kind="ExternalInput")
    dr["ob1"] = nc.dram_tensor("ob1", [P, 2], f32, kind="ExternalInput")
    dr["ow2"] = nc.dram_tensor("ow2", [P, 2, P], bf, kind="ExternalInput")
    dr["ob2"] = nc.dram_tensor("ob2", [1, 1], f32, kind="ExternalInput")
    out = nc.dram_tensor("out", [1, E_CORE], f32, kind="ExternalOutput")

    with tile.TileContext(nc) as tc:
        with (
            tc.tile_pool(name="const", bufs=1) as const,
            tc.tile_pool(name="mask", bufs=mbufs) as maskp,
            tc.tile_pool(name="prod", bufs=pbufs) as prodp,
            tc.tile_pool(name="sres", bufs=1) as sresp,
            tc.tile_pool(name="hid", bufs=2) as hidp,
            tc.tile_pool(name="psum_s", bufs=1, space="PSUM") as psum_s,
            tc.tile_pool(name="psum_m", bufs=4, space="PSUM") as psum_m,
        ):
            # one-time loads ride the scalar queue (masks own sync+gpsimd)
            x_sb = const.tile([P, KO, P], bf)
            nc.scalar.dma_start(x_sb[:], dr["xp"][:])
            w = {}
            for nm in ["c1w1", "c2w1", "c3w1", "c4w1", "ijw1"]:
                w[nm] = const.tile([P, HID], bf, name=nm)
                nc.scalar.dma_start(w[nm][:], dr[nm][:])
            for nm in ["c1w2", "c1w3", "c2w2", "c2w3", "c3w2", "c3w3",
                       "c4w2", "c4w3", "ijw2", "ow1", "ow2"]:
                w[nm] = const.tile([P, 2, HID if nm != "ow2" else P], bf,
                                   name=nm)
                nc.scalar.dma_start(w[nm][:], dr[nm][:])
            for nm in ["c1b1", "c1b2", "c1b3", "c2b1", "c2b2", "c2b3",
                       "c3b1", "c3b2", "c3b3", "c4b1", "c4b2", "c4b3",
                       "ijb1", "ijb2", "ob1"]:
                w[nm] = const.tile([P, 2], f32, name=nm)
                nc.scalar.dma_start(w[nm][:], dr[nm][:])
            w["ob2"] = const.tile([1, 1], f32, name="ob2")
            nc.scalar.dma_start(w["ob2"][:], dr["ob2"][:])
            xit_sb = const.tile([P, E_CORE], bf)
            nc.scalar.dma_start(xit_sb[:], dr["xit"][:])
            xjt_sb = const.tile([P, E_CORE], bf)
            nc.scalar.dma_start(xjt_sb[:], dr["xjt"][:])

            # stage 1: the four masked neighborhood sums. Mask chunks of 8
            # node-blocks (1 MiB transfers) alternate between the sync and
            # gpsimd DMA queues so transfers run on two queues in parallel.
            ps_s = [psum_s.tile([P, E_CORE], f32, name=f"s{b}")
                    for b in range(4)]
            for kl in range(KO // LCH):
                dma_eng = nc.sync if (kl % 2 == 0 or not dual_q) else nc.gpsimd
                mt = {}
                for nm in ["ait", "ajt", "a2it", "a2jt"]:
                    t = maskp.tile([P, LCH, E_CORE], bf, tag=nm,
                                   name=f"{nm}_{kl}")
                    src = dr[nm][kl * LCH * P:(kl + 1) * LCH * P, :]
                    dma_eng.dma_start(
                        t[:], src.rearrange("(c p) e -> p c e", p=P))
                    mt[nm] = t
                for pc in range(LCH // PCH):
                    sl = slice(pc * PCH, (pc + 1) * PCH)
                    prods = []
                    for b, (u, v) in enumerate([("ait", "ajt"),
                                                ("ait", "a2jt"),
                                                ("a2it", "ajt"),
                                                ("a2it", "a2jt")]):
                        pr = prodp.tile([P, PCH, E_CORE], bf, tag=f"m{b}",
                                        name=f"m{b}_{kl}_{pc}")
                        nc.vector.tensor_tensor(pr[:], mt[u][:, sl, :],
                                                mt[v][:, sl, :],
                                                mybir.AluOpType.mult)
                        prods.append(pr)
                    for c in range(PCH):
                        ko = kl * LCH + pc * PCH + c
                        for b in range(4):
                            nc.tensor.matmul(ps_s[b][:], x_sb[:, ko, :],
                                             prods[b][:, c, :],
                                             start=(ko == 0),
                                             stop=(ko == KO - 1))

            s_sb = [sresp.tile([P, E_CORE], bf, name=f"ssb{b}")
                    for b in range(4)]
            for b in range(4):
                nc.scalar.activation(s_sb[b][:], ps_s[b][:], F.Copy, bias=0.0)

            def act_relu(dst, ps, bias_ap, mo):
                # alternate ACT / DVE so the two engines split the MLP tail
                if mo == 0 or not tail_dve:
                    nc.scalar.activation(dst, ps, F.Relu, bias=bias_ap)
                else:
                    nc.vector.tensor_scalar(out=dst, in0=ps, scalar1=bias_ap,
                                            scalar2=0.0,
                                            op0=mybir.AluOpType.add,
                                            op1=mybir.AluOpType.max)

            def act_ident(dst, ps, bias_ap, mo):
                if mo == 0 or not tail_dve:
                    nc.scalar.activation(dst, ps, F.Identity, bias=bias_ap)
                else:
                    nc.vector.tensor_scalar(out=dst, in0=ps, scalar1=bias_ap,
                                            scalar2=None,
                                            op0=mybir.AluOpType.add)

            def mlp3(src, pre, out_name):
                h1 = hidp.tile([P, 2, E_CORE], bf, tag="h1", name="h1")
                for mo in range(2):
                    ps = psum_m.tile([P, E_CORE], f32, tag="mlp_ps",
                                     name="mlp_ps")
                    nc.tensor.matmul(ps[:],
                                     w[pre + "w1"][:, mo * P:(mo + 1) * P],
                                     src[:], start=True, stop=True)
                    act_relu(h1[:, mo, :], ps[:], w[pre + "b1"][:, mo:mo + 1],
                             mo)
                h2 = hidp.tile([P, 2, E_CORE], bf, tag="h2", name="h2")
                for mo in range(2):
                    ps = psum_m.tile([P, E_CORE], f32, tag="mlp_ps",
                                     name="mlp_ps")
                    for ki in range(2):
                        nc.tensor.matmul(
                            ps[:], w[pre + "w2"][:, ki, mo * P:(mo + 1) * P],
                            h1[:, ki, :], start=(ki == 0), stop=(ki == 1))
                    act_relu(h2[:, mo, :], ps[:], w[pre + "b2"][:, mo:mo + 1],
                             mo)
                h3 = hidp.tile([P, 2, E_CORE], bf, tag=out_name, name=out_name)
                for mo in range(2):
                    ps = psum_m.tile([P, E_CORE], f32, tag="mlp_ps",
                                     name="mlp_ps")
                    for ki in range(2):
                        nc.tensor.matmul(
                            ps[:], w[pre + "w3"][:, ki, mo * P:(mo + 1) * P],
                            h2[:, ki, :], start=(ki == 0), stop=(ki == 1))
                    act_ident(h3[:, mo, :], ps[:], w[pre + "b3"][:, mo:mo + 1],
                              mo)
                return h3

            xcn = [mlp3(s_sb[b], pre, f"xcn{b}")
                   for b, pre in enumerate(["c1", "c2", "c3", "c4"])]

            pij = prodp.tile([P, E_CORE], bf, tag="pij")
            nc.vector.tensor_tensor(pij[:], xit_sb[:], xjt_sb[:],
                                    mybir.AluOpType.mult)
            hij = hidp.tile([P, 2, E_CORE], bf, tag="hij")
            for mo in range(2):
                ps = psum_m.tile([P, E_CORE], f32, tag="mlp_ps", name="mlp_ps")
                nc.tensor.matmul(ps[:], w["ijw1"][:, mo * P:(mo + 1) * P],
                                 pij[:], start=True, stop=True)
                act_relu(hij[:, mo, :], ps[:], w["ijb1"][:, mo:mo + 1], mo)
            xij = hidp.tile([P, 2, E_CORE], bf, tag="xij")
            for mo in range(2):
                ps = psum_m.tile([P, E_CORE], f32, tag="mlp_ps", name="mlp_ps")
                for ki in range(2):
                    nc.tensor.matmul(ps[:],
                                     w["ijw2"][:, ki, mo * P:(mo + 1) * P],
                                     hij[:, ki, :], start=(ki == 0),
                                     stop=(ki == 1))
                act_ident(xij[:, mo, :], ps[:], w["ijb2"][:, mo:mo + 1], mo)

            z = hidp.tile([P, 2, E_CORE], bf, tag="z")
            nc.vector.tensor_tensor(z[:], xcn[1][:], xcn[2][:],
                                    mybir.AluOpType.mult)
            nc.vector.tensor_tensor(z[:], z[:], xcn[0][:],
                                    mybir.AluOpType.add)
            nc.vector.tensor_tensor(z[:], z[:], xcn[3][:],
                                    mybir.AluOpType.add)
            nc.vector.tensor_tensor(z[:], z[:], xij[:], mybir.AluOpType.add)

            ho = hidp.tile([P, 2, E_CORE], bf, tag="ho")
            for mo in range(2):
                ps = psum_m.tile([P, E_CORE], f32, tag="mlp_ps", name="mlp_ps")
                for ki in range(2):
                    nc.tensor.matmul(ps[:], w["ow1"][:, ki, mo * P:(mo + 1) * P],
                                     z[:, ki, :], start=(ki == 0),
                                     stop=(ki == 1))
                act_relu(ho[:, mo, :], ps[:], w["ob1"][:, mo:mo + 1], mo)
            ps = psum_m.tile([P, E_CORE], f32, tag="mlp_ps", name="mlp_ps")
            for ki in range(2):
                nc.tensor.matmul(ps[:], w["ow2"][:, ki, :], ho[:, ki, :],
                                 start=(ki == 0), stop=(ki == 1))
            out_sb = sresp.tile([1, E_CORE], f32, name="out_sb")
            nc.scalar.activation(out_sb[:], ps[0:1, :], F.Identity,
                                 bias=w["ob2"][:])
            nc.sync.dma_start(out[:], out_sb[:])

    nc.compile()
    _BUILD_CACHE[key] = nc
    return nc, key


def _mlp3_weights(pre, params, scale3=None):
    W1, b1, W2, b2, W3, b3 = [np.asarray(t, np.float32) for t in params]
    if scale3 is not None:
        W3 = W3 * scale3
        b3 = b3 * scale3
    out = {}
    out[pre + "w1"] = np.ascontiguousarray(W1.astype(BF16))
    out[pre + "w2"] = np.ascontiguousarray(
        W2.reshape(2, P, HID).transpose(1, 0, 2).astype(BF16))
    out[pre + "w3"] = np.ascontiguousarray(
        W3.reshape(2, P, HID).transpose(1, 0, 2).astype(BF16))
    out[pre + "b1"] = np.ascontiguousarray(b1.reshape(2, P).T.astype(np.float32))
    out[pre + "b2"] = np.ascontiguousarray(b2.reshape(2, P).T.astype(np.float32))
    out[pre + "b3"] = np.ascontiguousarray(b3.reshape(2, P).T.astype(np.float32))
    return out


def kernel(x, adj, tar_ei, alpha, beta, p_cn1, p_cn2, p_cn4, p_ij, p_out):
    x = np.asarray(x, np.float32)
    adj = np.asarray(adj, np.float32)
    tar = np.asarray(tar_ei).astype(np.int64)
    assert x.shape == (N, IN_CH) and adj.shape == (N, N)
    assert tar.shape == (2, E)
    cores = list(range(NCORES))

    # ---------------- phase A: adj2 rows for unique endpoints ----------------
    # A2[u] = OR over v in nbr(u) of adj[v], on bit-packed rows. The host
    # gathers the needed packed rows per shard (data distribution); the
    # device does the OR reduction.
    uniq, inv = np.unique(tar, return_inverse=True)
    inv = inv.reshape(tar.shape)
    U = uniq.size
    NB = int(np.ceil(U / (NCORES * P)))
    U_pad = NB * P * NCORES

    adj_u8 = (adj != 0).astype(np.uint8)
    adjP = np.packbits(adj_u8, axis=1)
    adjP = np.vstack([adjP, np.zeros((1, N // 8), np.uint8)])  # zero row at N
    adjP32 = np.ascontiguousarray(adjP).view(np.uint32)

    sub = adj_u8[uniq]
    deg = sub.sum(1).astype(np.int64)
    degp = np.concatenate([deg, np.zeros(U_pad - U, np.int64)])
    order = np.argsort(degp, kind="stable")
    assign = order.reshape(NB * P, NCORES).T  # [core, pos] -> padded slot

    rows, cols = np.nonzero(sub)
    maxdeg = max(int(deg.max()), 1)
    starts = np.zeros(U + 1, np.int64)
    np.cumsum(deg, out=starts[1:])
    rank = np.arange(len(cols)) - starts[rows]
    nbrmat = np.full((U_pad, maxdeg), N, np.int64)
    nbrmat[rows, rank] = cols

    degp_sorted = degp[order]
    steps = tuple(
        max(int(degp_sorted[(b + 1) * P * NCORES - 1]), 1) for b in range(NB))

    in_maps_a = []
    for c in cores:
        blocks = []
        for b in range(NB):
            nodes = assign[c, b * P:(b + 1) * P]
            blocks.append(nbrmat[nodes, :steps[b]].T)
        I = np.concatenate(blocks, axis=0)
        in_maps_a.append({"gath": np.ascontiguousarray(adjP32[I])})

    nc_a, akey = _build_phase_a2(NB, steps)
    _LAST_IN_MAPS.clear()
    _LAST_IN_MAPS[akey] = in_maps_a
    res_a = run_bass_kernel_spmd(nc_a, in_maps_a, cores)

    a2_pad = np.empty((U_pad, W32), np.uint32)
    for c in cores:
        a2_pad[assign[c]] = res_a.results[c]["a2p"]
    a2_rows = np.unpackbits(a2_pad[:U].view(np.uint8), axis=1).astype(BF16)

    # ---------------- host: per-edge gathers (sharding) ----------------------
    i_all, j_all = tar[0], tar[1]
    inv_i, inv_j = inv[0], inv[1]
    adj_bf = adj_u8.astype(BF16)
    x_bf = x.astype(BF16)
    xp = np.ascontiguousarray(x_bf.reshape(KO, P, P).transpose(1, 0, 2))

    alpha = np.asarray(alpha, np.float64).reshape(3)
    beta_v = float(np.asarray(beta, np.float64).reshape(1)[0])
    a = np.cumprod(1.0 / (1.0 + np.exp(-alpha))).astype(np.float32)

    wmaps = {}
    wmaps.update(_mlp3_weights("c1", p_cn1, a[0]))
    wmaps.update(_mlp3_weights("c2", p_cn2, a[1]))
    wmaps.update(_mlp3_weights("c3", p_cn2, None))
    wmaps.update(_mlp3_weights("c4", p_cn4, a[2]))
    Wij1, bij1, Wij2, bij2 = [np.asarray(t, np.float32) for t in p_ij]
    Wij2 = Wij2 * beta_v
    bij2 = bij2 * beta_v
    wmaps["ijw1"] = np.ascontiguousarray(Wij1.astype(BF16))
    wmaps["ijw2"] = np.ascontiguousarray(
        Wij2.reshape(2, P, HID).transpose(1, 0, 2).astype(BF16))
    wmaps["ijb1"] = np.ascontiguousarray(bij1.reshape(2, P).T.astype(np.float32))
    wmaps["ijb2"] = np.ascontiguousarray(bij2.reshape(2, P).T.astype(np.float32))
    Wo1, bo1, Wo2, bo2 = [np.asarray(t, np.float32) for t in p_out]
    wmaps["ow1"] = np.ascontiguousarray(
        Wo1.reshape(2, P, HID).transpose(1, 0, 2).astype(BF16))
    wmaps["ob1"] = np.ascontiguousarray(bo1.reshape(2, P).T.astype(np.float32))
    ow2 = np.zeros((2, P, P), np.float32)
    ow2[:, :, 0] = Wo2.reshape(2, P)
    wmaps["ow2"] = np.ascontiguousarray(ow2.transpose(1, 0, 2).astype(BF16))
    wmaps["ob2"] = np.full((1, 1), np.float32(bo2.reshape(1)[0]), np.float32)

    in_maps_b = []
    for c in cores:
        sl = slice(c * E_CORE, (c + 1) * E_CORE)
        i_s, j_s = i_all[sl], j_all[sl]
        m = dict(wmaps)
        m["ait"] = np.ascontiguousarray(adj_bf[i_s].T)
        m["ajt"] = np.ascontiguousarray(adj_bf[j_s].T)
        m["a2it"] = np.ascontiguousarray(a2_rows[inv_i[sl]].T)
        m["a2jt"] = np.ascontiguousarray(a2_rows[inv_j[sl]].T)
        m["xp"] = xp
        m["xit"] = np.ascontiguousarray(x_bf[i_s].T)
        m["xjt"] = np.ascontiguousarray(x_bf[j_s].T)
        in_maps_b.append(m)

    nc_b, bkey = _build_phase_b()
    _LAST_IN_MAPS[bkey] = in_maps_b
    res_b = run_bass_kernel_spmd(nc_b, in_maps_b, cores)
    out = np.concatenate(
        [np.asarray(res_b.results[c]["out"][0], np.float32) for c in cores])
    return out[:, None]
